# revision 32
# baseline (speedup 1.0000x reference)
"""Trainium2 Bass kernel for nn_CalibrationNetwork (MoE-routed 3-layer MLP + softmax).

Strategy (v3): judge-contiguous scheduling
------------------------------------------
Host sorts samples by judge. The 32 judges are ranked by size and snake-
assigned to 8 cores x 4 slots; slot k has a common (max-padded) pair count
across cores so one SPMD program serves all cores. Per-judge combined
weights (W1+W1_a etc.) are packed per slot; all matmuls then use N=512
moving columns so LDWEIGHTS hides in the PE reorder window.

Layouts (parity-pair packing, 2 samples per column everywhere):
  z partition   p = par*64 + h           (par = sample parity in its pair)
  L1: stationary per q = [12 rows=(par,d0..5), 128 cols=(par,h)] block-diag,
      7 q's stacked on disjoint row ranges QROW[q] (2 per 32-strip) so all
      share one 128-col block; x lives on the same rows. Row-tiled matmuls
      (tile_position=(32g,0)) run concurrently across strips.
  L2: stationary [128=(par,h1), 128=(par,h2)] block-diag; bias b2 applied
      by the ACT relu2 drain (f32 bias columns).
  L3: stationary per q = V block [128=(par,h2), 10=(par,o)], col-tiled
      (tile_position=(0,32g)); output partitions 32g+par*5+o.
  V/b3 bias and softmax are applied on the HOST (judge known per sample).

Pipeline: chunks of <=512 pairs flow L1 -> relu1(DVE) -> L2 -> relu2(ACT)
-> L3 -> copy(ACT) with software-pipelined emission (L1 of chunk i is
emitted with L2 of chunk i-1 and L3 of chunk i-2) and two 2-bank PSUM
tags as the pipeline buffer.  In the last two chunks the relu2/copy
drains move to the DVE (chained max/add with per-partition -b2/+b2
scalars) so the pipeline drain-out is not serialized on the ACT engine.
Inputs arrive as one combined [wt | xA | xB] block per slot (4 DMAs,
slot 0 first) since each HWDGE dma_start costs ~0.6us of issue time;
the ACT Relu table is preloaded while DMAs land.  PE runs at 1.2 GHz
throughout on this part (HAM never unthrottles), which makes the PE the
critical path at ~30us busy; DVE/ACT PSUM drains are ~19/24us.
"""

import numpy as np
import ml_dtypes

B, J, Q, O, H = 32768, 32, 7, 5, 64
N_CORES = 8
NSLOTS = J // N_CORES          # 4 judges (slots) per core
CHUNK = 512                    # max pairs per matmul
PADP = 32                      # slot pair counts padded to multiple of this
WJ = 458                       # weight cols/slot: 2x128 L1+128 L2+70 V+2 b2+2 negb2
QROW = (0, 32, 64, 96, 0, 32, 64)    # x/W1 row base per question (32-aligned)
XB_OF_Q = (0, 0, 0, 0, 1, 1, 1)      # x/W1 column block per question
G_OF_Q = (0, 1, 2, 3, 0, 1, 2)       # 32-strip (row/col group) per question

_bf16 = ml_dtypes.bfloat16
_cache = {}


def _chunks(S, ragged_first=False):
    """Split S pairs into matmul chunks (512s + one ragged multiple of 32)."""
    out = [CHUNK] * (S // CHUNK)
    if S % CHUNK:
        if ragged_first:
            out = [S % CHUNK] + out
        else:
            out.append(S % CHUNK)
    return out


def _chunk_plan(slots):
    """Per-slot chunk lists (fewest chunks: 512s plus one ragged tail)."""
    return [_chunks(S) for S in slots]


# ----------------------------------------------------------------------------
# device program
# ----------------------------------------------------------------------------

def _build_program(slots):
    import concourse.bacc as bacc
    import concourse.tile as tile
    import concourse.mybir as mybir
    import concourse.bass as bass
    from contextlib import ExitStack

    bf = mybir.dt.bfloat16
    f32 = mybir.dt.float32
    AF = mybir.ActivationFunctionType

    TP = sum(slots)
    offs = np.cumsum([0] + list(slots))[:-1]

    nc = bacc.Bacc("TRN2", target_bir_lowering=False, debug=False)
    IN_COLS = NSLOTS * WJ + 2 * TP
    in_d = nc.dram_tensor("inb", (128, IN_COLS), bf, kind="ExternalInput")
    out_d = nc.dram_tensor("out", (128, 2 * TP), bf, kind="ExternalOutput")
    # per-slot base col in the combined input block
    cb = [int(sum(WJ + 2 * slots[t] for t in range(s))) for s in range(NSLOTS)]

    with ExitStack() as ctx:
        tc = ctx.enter_context(tile.TileContext(nc))
        cpool = ctx.enter_context(tc.tile_pool(name="const", bufs=1))
        ppool = ctx.enter_context(tc.tile_pool(name="ps", bufs=2, space="PSUM"))

        in_t = cpool.tile([128, IN_COLS], bf)
        z1 = cpool.tile([128, 7 * TP], bf)
        z2 = cpool.tile([128, 7 * TP], bf)
        lg = cpool.tile([128, 2 * TP], bf)
        warm = cpool.tile([1, 8], bf)

        nc.vector.memset(warm[:], 1.0)

        # one combined [wt | xaA | xaB] DMA per slot, slot 0 first
        for s in range(NSLOTS):
            w = WJ + 2 * slots[s]
            nc.sync.dma_start(in_t[:, cb[s]:cb[s] + w],
                              in_d.ap()[:, cb[s]:cb[s] + w])
        # preload the ACT Relu table while DMAs land
        nc.scalar.activation(warm[0:1, 0:1], warm[0:1, 1:2], AF.Relu, scale=1.0)

        # chunk sequence: (slot, pair0, npairs)
        seq = []
        plan = _chunk_plan(slots)
        for s in range(NSLOTS):
            p0 = 0
            for n in plan[s]:
                seq.append((s, p0, n))
                p0 += n

        def drain3(engine_op, t, nb, n, dst, **kw):
            """Drain nb banks of n cols each from tile t into contiguous dst."""
            if n == 512:
                engine_op(dst, t[:, 0:nb * 512], **kw)
            else:
                src = bass.AP(t[:].tensor, t[:].offset,
                              [list(t[:].ap[0]), [512, nb], [1, n]])
                d = bass.AP(dst.tensor, dst.offset,
                            [list(dst.ap[0]), [n, nb], [1, n]])
                engine_op(d, src, **kw)

        def l1mm(i, t, q, bank):
            s, p0, n = seq[i]
            rw = QROW[q]
            xb = XB_OF_Q[q]
            xc = cb[s] + WJ
            nc.tensor.matmul(
                t[:, 512 * bank:512 * bank + n],
                in_t[rw:rw + 12, cb[s] + 128 * xb:cb[s] + 128 * xb + 128],
                in_t[rw:rw + 12, xc + xb * slots[s] + p0:xc + xb * slots[s] + p0 + n],
                start=True, stop=True,
                tile_position=(32 * G_OF_Q[q], 0))

        def l1drain(i, t, bank0, nb, qslot0):
            s, p0, n = seq[i]
            zb = 7 * int(offs[s]) + 7 * p0
            if n == 512:
                src_ap = t[:, 512 * bank0:512 * (bank0 + nb)]
                dst = z1[:, zb + qslot0 * n:zb + (qslot0 + nb) * n]
            else:
                base = t[:, 512 * bank0:512 * (bank0 + nb)]
                src_ap = bass.AP(base.tensor, base.offset,
                                 [list(base.ap[0]), [512, nb], [1, n]])
                d = z1[:, zb + qslot0 * n:zb + (qslot0 + nb) * n]
                src_ap = src_ap
                dst = bass.AP(d.tensor, d.offset,
                              [list(d.ap[0]), [n, nb], [1, n]])
            nc.vector.tensor_scalar_max(dst, src_ap, 0.0)

        def l1a(i):
            # q0-3 into a fresh 4-bank tile, 4-concurrent across row strips
            t = ppool.tile([128, 2048], f32, tag="pa", name=f"p1_{i}", bufs=1)
            for k in range(4):
                l1mm(i, t, k, k)
            l1drain(i, t, 0, 2, 0)      # q0,q1
            l1drain(i, t, 2, 2, 2)      # q2,q3
            return t

        def l1b(i, t):
            # q4-6 reuse banks 0-2 once their half-drain completes
            for k, q in enumerate((4, 5, 6)):
                l1mm(i, t, q, k)
            l1drain(i, t, 0, 2, 4)      # q4,q5
            l1drain(i, t, 2, 1, 6)      # q6

        def relu2_drain(src_ap, dst_ap, wc, on_dve):
            if on_dve:
                nb2 = in_t[:, wc + 456:wc + 458].bitcast(f32)
                b2 = in_t[:, wc + 454:wc + 456].bitcast(f32)
                nc.vector.tensor_scalar(dst_ap, src_ap, nb2, b2,
                                        mybir.AluOpType.max, mybir.AluOpType.add)
            else:
                b2ap = in_t[:, wc + 454:wc + 456].bitcast(f32)
                nc.scalar.activation(dst_ap, src_ap, AF.Relu, bias=b2ap, scale=1.0)

        def l2piece(i, p, on_dve=False):
            """Emit the p-th 1024-col piece of chunk i's L2, if it exists."""
            s, p0, n = seq[i]
            done = 1024 * p
            if done >= 7 * n:
                return
            piece = min(1024, 7 * n - done)
            o = int(offs[s])
            wc = cb[s]
            zb = 7 * o + 7 * p0
            t = ppool.tile([128, 1024], f32, tag="pb", name=f"p2_{i}_{p}")
            na = min(512, piece)
            nc.tensor.matmul(t[:, 0:na], in_t[:, wc + 256:wc + 384],
                             z1[:, zb + done:zb + done + na],
                             start=True, stop=True)
            if piece > 512:
                nc.tensor.matmul(t[:, 512:piece], in_t[:, wc + 256:wc + 384],
                                 z1[:, zb + done + 512:zb + done + piece],
                                 start=True, stop=True)
            if piece <= 512 or piece == 1024:
                relu2_drain(t[:, 0:piece], z2[:, zb + done:zb + done + piece],
                            wc, on_dve)
            else:
                relu2_drain(t[:, 0:512], z2[:, zb + done:zb + done + 512],
                            wc, on_dve)
                relu2_drain(t[:, 512:piece],
                            z2[:, zb + done + 512:zb + done + piece], wc, on_dve)

        def l3(i, on_dve=False):
            s, p0, n = seq[i]
            o = int(offs[s])
            wc = cb[s]
            zb = 7 * o + 7 * p0
            lgb = 2 * o + 2 * p0
            t = ppool.tile([128, 1024], f32, tag="pb", name=f"p3_{i}")
            for q in range(7):
                g = G_OF_Q[q]
                rnd = 0 if q < 4 else 1
                nc.tensor.matmul(
                    t[32 * g:32 * g + 10, 512 * rnd:512 * rnd + n],
                    in_t[:, wc + 384 + 10 * q:wc + 394 + 10 * q],
                    z2[:, zb + q * n:zb + (q + 1) * n],
                    start=True, stop=True,
                    tile_position=(0, 32 * g))
            cp = nc.vector.tensor_copy if on_dve else nc.scalar.copy
            drain3(cp, t, 2, n, lg[:, lgb:lgb + 2 * n])
            # store once per slot, when its last chunk drains
            if i == len(seq) - 1 or seq[i + 1][0] != s:
                eng = nc.sync if i == len(seq) - 1 else nc.gpsimd
                eng.dma_start(out_d.ap()[:, 2 * o:2 * o + 2 * slots[s]],
                              lg[:, 2 * o:2 * o + 2 * slots[s]])

        # software-pipelined emission: L1(i) | L2(i-1) | L3(i-2).  In the
        # last chunks the relu2/copy drains move to the DVE so the pipeline
        # drain-out is not serialized on the ACT engine.
        nseq = len(seq)
        for i in range(nseq + 2):
            cur, prev = i < nseq, 1 <= i <= nseq
            tail2 = nseq - (i - 1) <= 2        # chunk i-1 is in the last two
            tail3 = nseq - (i - 2) <= 2
            t = l1a(i) if cur else None
            if prev:
                l2piece(i - 1, 0)
                l2piece(i - 1, 1, on_dve=tail2)
            if t is not None:
                l1b(i, t)
            if prev:
                l2piece(i - 1, 2)
                l2piece(i - 1, 3, on_dve=tail2)
            if 2 <= i:
                l3(i - 2, on_dve=tail3)

    nc.compile()
    return nc


def _get_program(slots):
    key = tuple(slots)
    if key not in _cache:
        _cache[key] = _build_program(key)
    return _cache[key]


# ----------------------------------------------------------------------------
# host-side scheduling and packing
# ----------------------------------------------------------------------------

def _schedule(judge_ids):
    """Snake-assign judges to (core, slot); returns slot sizes + per-core judge
    lists + per-judge sample index arrays (sorted order)."""
    jid = np.asarray(judge_ids).astype(np.int64).ravel()
    assert jid.shape[0] == B
    order = np.argsort(jid, kind="stable")
    counts = np.bincount(jid, minlength=J)
    pos = np.cumsum([0] + list(counts))
    samples = [order[pos[j]:pos[j + 1]] for j in range(J)]
    pairs = np.array([(c + 1) // 2 for c in counts])

    rank = np.argsort(-pairs, kind="stable")
    slots = []
    assign = np.zeros((N_CORES, NSLOTS), np.int64)   # judge id per (core, slot)
    for s in range(NSLOTS):
        grp = rank[8 * s:8 * s + 8]
        size = int(-(-max(1, pairs[grp].max()) // PADP) * PADP)
        slots.append(size)
        for k in range(N_CORES):
            assign[k, s] = grp[k]
    return tuple(slots), assign, samples, pairs


def _pack_inputs(x, judge_ids, W1c, W2c, Vc, slots, assign, samples):
    TP = sum(slots)
    offs = np.cumsum([0] + list(slots))[:-1]
    x = np.asarray(x, np.float32)

    # weights per judge, packed per (core, slot)
    wtj = np.zeros((J, 128, WJ), np.float32)
    for q in range(Q):
        rw = QROW[q]
        xb = XB_OF_Q[q]
        for par in range(2):
            # L1 block: rows rw+par*6+d, cols 128*xb + par*64+h
            blk = W1c[:, q].transpose(0, 2, 1)          # [J, d, h]
            wtj[:, rw + 6 * par:rw + 6 * par + 6,
                128 * xb + 64 * par:128 * xb + 64 * par + 64] = blk
    for par in range(2):
        s = slice(64 * par, 64 * par + 64)
        wtj[:, s, 256 + 64 * par:256 + 64 * par + 64] = \
            W2c[:, :, 1:].transpose(0, 2, 1)            # rows h1, cols h2
        for q in range(Q):
            wtj[:, s, 384 + 10 * q + 5 * par:384 + 10 * q + 5 * par + 5] = \
                Vc[:, q, :, 1:].transpose(0, 2, 1)      # rows h2, cols o
    wt16 = np.zeros((J, 128, WJ), np.uint16)
    wt16[:, :, :454] = wtj[:, :, :454].astype(_bf16).view(np.uint16)
    b2 = np.concatenate([W2c[:, :, 0], W2c[:, :, 0]], axis=1)   # [J, 128]
    wt16[:, :, 454:456] = b2.astype(np.float32).view(np.uint16).reshape(J, 128, 2)
    wt16[:, :, 456:458] = (-b2).astype(np.float32).view(np.uint16).reshape(J, 128, 2)

    cb = [int(sum(WJ + 2 * slots[t] for t in range(s))) for s in range(NSLOTS)]
    IN_COLS = NSLOTS * WJ + 2 * TP
    in_maps = []
    for k in range(N_CORES):
        inb = np.zeros((128, IN_COLS), np.uint16)
        for s in range(NSLOTS):
            j = int(assign[k, s])
            g = samples[j]
            S = slots[s]
            inb[:, cb[s]:cb[s] + WJ] = wt16[j]
            xa = np.zeros((128, 2 * S), np.float32)
            for par in range(2):
                gs = g[par::2]
                ns = len(gs)
                xv = x[gs]                               # [ns, Q, O]
                for q in range(Q):
                    rw = QROW[q] + 6 * par
                    c0 = XB_OF_Q[q] * S
                    xa[rw, c0:c0 + ns] = 1.0
                    xa[rw + 1:rw + 6, c0:c0 + ns] = xv[:, q, :].T
            inb[:, cb[s] + WJ:cb[s] + WJ + 2 * S] = \
                xa.astype(_bf16).view(np.uint16)
        in_maps.append({"inb": inb.view(_bf16)})
    return in_maps


def _unpack_output(results, judge_ids, b3, slots, assign, samples):
    TP = sum(slots)
    offs = np.cumsum([0] + list(slots))[:-1]
    logits = np.zeros((B, Q, O), np.float32)
    for k in range(N_CORES):
        blob = np.asarray(results[k]["out"]).view(np.uint16)
        f = (blob.astype(np.uint32) << 16).view(np.float32)   # [128, 2*TP]
        for s in range(NSLOTS):
            j = int(assign[k, s])
            o = int(offs[s])
            g = samples[j]
            S = len(g)
            p0 = 0
            for n in _chunk_plan(slots)[s]:
                for q in range(Q):
                    gq = G_OF_Q[q]
                    rnd = 0 if q < 4 else 1
                    cb = 2 * o + 2 * p0 + rnd * n
                    for par in range(2):
                        rows = slice(32 * gq + 5 * par, 32 * gq + 5 * par + 5)
                        idx = 2 * (p0 + np.arange(n)) + par
                        v = idx < S
                        if v.any():
                            logits[g[idx[v]], q, :] = f[rows, cb:cb + n].T[v]
                p0 += n
    logits += b3[np.asarray(judge_ids).astype(np.int64).ravel()]
    m = logits.max(-1, keepdims=True)
    e = np.exp(logits - m)
    return (e / e.sum(-1, keepdims=True)).astype(np.float32)


def _prepare(x, judge_ids, W1, W1_a, W2, W2_a, V, V_a):
    W1c = (np.asarray(W1, np.float32)[None] + np.asarray(W1_a, np.float32))
    W2c = (np.asarray(W2, np.float32)[None] + np.asarray(W2_a, np.float32))
    Vc = (np.asarray(V, np.float32)[None] + np.asarray(V_a, np.float32))
    b3 = Vc[:, :, :, 0]                                  # [J, Q, O]
    slots, assign, samples, pairs = _schedule(judge_ids)
    in_maps = _pack_inputs(x, judge_ids, W1c, W2c, Vc, slots, assign, samples)
    return in_maps, (judge_ids, b3, slots, assign, samples)


# ----------------------------------------------------------------------------
# entry points
# ----------------------------------------------------------------------------

def kernel(x, judge_ids, W1, W1_a, W2, W2_a, V, V_a):
    from concourse import bass_utils
    in_maps, meta = _prepare(x, judge_ids, W1, W1_a, W2, W2_a, V, V_a)
    nc = _get_program(meta[2])
    res = bass_utils.run_bass_kernel_spmd(
        nc, in_maps, core_ids=list(range(N_CORES)), trace=False)
    return _unpack_output(res.results, meta[0], meta[1], meta[2], meta[3], meta[4])


def run_with_results(x, judge_ids, W1, W1_a, W2, W2_a, V, V_a, trace=False,
                     **kwargs):
    from concourse import bass_utils
    in_maps, meta = _prepare(x, judge_ids, W1, W1_a, W2, W2_a, V, V_a)
    nc = _get_program(meta[2])
    res = bass_utils.run_bass_kernel_spmd(
        nc, in_maps, core_ids=list(range(N_CORES)), trace=trace, **kwargs)
    out = _unpack_output(res.results, meta[0], meta[1], meta[2], meta[3], meta[4])
    return out, res
